# revision 35
# baseline (speedup 1.0000x reference)
"""Trainium2 Bass kernel for nn_Actor (Hopf-oscillator CPG actor network).

Strategy
--------
The 100-step recurrence only advances z via the cheap elementwise Hopf update;
the 3-layer complex MLP applied at each step does NOT feed back into the
recurrence.  So per core (data-parallel batch shard of 32):

  1. Encoder (exact fp32 matmuls, feature-major layout, tiny).
  2. Sequentially generate z_1..z_104 on the Vector/Scalar engines
     (state kept feature-major, packed flat [128, 256] = [x-chunks | y-chunks]).
  3. Run the complex MLP over groups of 8 steps (256 rows) as large
     weight-stationary float32r matmuls on the Tensor engine.  Complex
     products are accumulated directly in PSUM using pre-negated weight
     copies (yr = Wr@x + (-Wi)@y, yi = Wi@x + Wr@y), so the combine is a
     single ScalarE relu+bias from PSUM (layer 1 uses the 3-multiply
     Karatsuba form with VectorE combines instead).  Only the real part of
     layer 2 is computed (the reference keeps out[:, :, :256]).
  4. Results are DMA'd out feature-major; the host transposes to
     [B, STEPS, A] while unsharding.

All 8 cores run the same SPMD graph on their own batch shard; weights are
replicated; no collectives.  abs(relu(x)) == relu(x) so the encoder's abs()
is dropped.
"""

import os
import sys

import numpy as np

for _p in ("/opt/trn_rl_repo", "/opt/pypackages"):
    if os.path.isdir(_p) and _p not in sys.path:
        sys.path.append(_p)

B = 256
NCORES = 8
BS = B // NCORES          # 32 batch rows per core
U = 512                   # oscillator units
STEPS = 100
DT = 0.001
A = 256                   # action dim

G_STEPS = 8               # steps per MLP group
G = G_STEPS * BS          # 256 rows per group (moving dim; >=256 keeps f32r fast)
NG = 13                   # 13 * 8 = 104 generated states (z_1..z_104, last 4 pad)

_CACHE = {}


def _build():
    from contextlib import ExitStack

    from concourse import bacc, masks, mybir

    from concourse.tile import TileContext

    f32 = mybir.dt.float32
    f32r = mybir.dt.float32r
    i32 = mybir.dt.int32
    AF = mybir.ActivationFunctionType
    OP = mybir.AluOpType

    nc = bacc.Bacc("TRN2", target_bir_lowering=False, debug=False,
                   num_devices=NCORES)

    def dp(name, shape, out=False):
        return nc.declare_dram_parameter(name, list(shape), f32, isOutput=out)

    motion_ext = dp("motion_state", [BS, 64])
    robot_ext = dp("robot_state", [BS, 128])
    z_ext = dp("z", [BS, 2 * U])
    W_ms_ext = dp("W_ms", [64, 256]); b_ms_ext = dp("b_ms", [256])
    W_rs_ext = dp("W_rs", [128, 256]); b_rs_ext = dp("b_rs", [256])
    W_cmb_ext = dp("W_cmb", [512, 512]); b_cmb_ext = dp("b_cmb", [512])
    W_om_ext = dp("W_om", [512, 1]); b_om_ext = dp("b_om", [1])
    W_mu_ext = dp("W_mu", [512, 512]); b_mu_ext = dp("b_mu", [512])
    W_b_ext = dp("W_b", [512, 512]); b_b_ext = dp("b_b", [512])
    Wr0_ext = dp("Wr0", [512, 1024]); Wi0_ext = dp("Wi0", [512, 1024])
    br0_ext = dp("br0", [1024]); bi0_ext = dp("bi0", [1024])
    Wr1_ext = dp("Wr1", [1024, 512]); Wi1_ext = dp("Wi1", [1024, 512])
    br1_ext = dp("br1", [512]); bi1_ext = dp("bi1", [512])
    Wr2_ext = dp("Wr2", [512, 256]); Wi2_ext = dp("Wi2", [512, 256])
    br2_ext = dp("br2", [256]); bi2_ext = dp("bi2", [256])

    # feature-major, group-major output; host transposes during unshard
    out_ext = dp("out_fm", [NG, 128, 2, G], out=True)
    zout_ext = dp("z_out", [BS, 2 * U], out=True)
    omega_ext = dp("omega", [BS, 1], out=True)
    mu_ext = dp("mu", [BS, U], out=True)
    bb_ext = dp("bb", [BS, U], out=True)

    with TileContext(nc) as tc, ExitStack() as ctx:
        # ---------- persistent pools ----------
        wp = ctx.enter_context(tc.tile_pool(name="wpool", bufs=1))
        stp = ctx.enter_context(tc.tile_pool(name="stpool", bufs=2))

        ident = wp.tile([128, 128], f32, tag="ident", name="ident")
        masks.make_identity(nc, ident[:, :])

        # MLP weights (float32r, cast on DMA by gpsimd) + negated Wi copies
        def wload(ext, n_k, width, base):
            ts = []
            for k in range(n_k):
                t = wp.tile([128, width], f32r, tag=f"{base}{k}", name=f"{base}{k}")
                nc.gpsimd.dma_start(out=t[:, :], in_=ext[k * 128:(k + 1) * 128, :])
                ts.append(t)
            return ts

        wr0 = wload(Wr0_ext, 4, 1024, "wr0_")
        wi0 = wload(Wi0_ext, 4, 1024, "wi0_")
        wr1 = wload(Wr1_ext, 8, 512, "wr1_")
        wi1 = wload(Wi1_ext, 8, 512, "wi1_")
        wr2 = wload(Wr2_ext, 4, 256, "wr2_")
        wi2 = wload(Wi2_ext, 4, 256, "wi2_")
        nwi0, nwi2 = [], []
        for k in range(4):
            t = wp.tile([128, 1024], f32r, tag=f"nwi0_{k}", name=f"nwi0_{k}")
            nc.vector.tensor_scalar_mul(t[:, :], wi0[k][:, :], -1.0)
            nwi0.append(t)
        wq1 = []
        for k in range(8):
            t = wp.tile([128, 512], f32r, tag=f"wq1_{k}", name=f"wq1_{k}")
            nc.vector.tensor_add(t[:, :], wr1[k][:, :], wi1[k][:, :])
            wq1.append(t)
        for k in range(4):
            t = wp.tile([128, 256], f32r, tag=f"nwi2_{k}", name=f"nwi2_{k}")
            nc.vector.tensor_scalar_mul(t[:, :], wi2[k][:, :], -1.0)
            nwi2.append(t)

        # biases, partition-major: tile[p, o] = b[o*128 + p]
        def bias_tile(ext, n_chunks, name, eng):
            t = wp.tile([128, n_chunks], f32, tag=name, name=name)
            eng.dma_start(out=t[:, :],
                          in_=ext[:].rearrange("(o p) -> p o", p=128))
            return t

        br0_t = bias_tile(br0_ext, 8, "br0_t", nc.sync)
        bi0_t = bias_tile(bi0_ext, 8, "bi0_t", nc.sync)
        br1_t = bias_tile(br1_ext, 4, "br1_t", nc.sync)
        bi1_t = bias_tile(bi1_ext, 4, "bi1_t", nc.sync)
        br2_t = bias_tile(br2_ext, 2, "br2_t", nc.sync)
        b_ms_t = bias_tile(b_ms_ext, 2, "b_ms_t", nc.sync)
        b_rs_t = bias_tile(b_rs_ext, 2, "b_rs_t", nc.sync)
        b_cmb_t = bias_tile(b_cmb_ext, 4, "b_cmb_t", nc.sync)
        b_mu_t = bias_tile(b_mu_ext, 4, "b_mu_t", nc.sync)
        b_b_t = bias_tile(b_b_ext, 4, "b_b_t", nc.sync)
        b_om_t = wp.tile([1, 1], f32, tag="b_om_t", name="b_om_t")
        nc.sync.dma_start(out=b_om_t[:, :],
                          in_=b_om_ext[:].rearrange("(a b) -> a b", b=1))

        # hopf constants (flat [128, 128]: col = chunk*32 + batch)
        one_dtmu = wp.tile([128, 128], f32, tag="one_dtmu", name="one_dtmu")
        dtbb = wp.tile([128, 128], f32, tag="dtbb", name="dtbb")
        dtw = wp.tile([128, 128], f32, tag="dtw", name="dtw")

        # ---------- encoder (scratch pools, freed before the main loop) ----
        with tc.tile_pool(name="encp", bufs=1) as ep, \
             tc.tile_pool(name="encps", bufs=2, space="PSUM") as eps:

            # inputs first (everything downstream depends on them)
            min_sb = ep.tile([BS, 64], f32, tag="min_sb", name="min_sb")
            nc.sync.dma_start(out=min_sb[:, :], in_=motion_ext[:, :])
            rin_sb = ep.tile([BS, 128], f32, tag="rin_sb", name="rin_sb")
            nc.sync.dma_start(out=rin_sb[:, :], in_=robot_ext[:, :])
            zin_sb = ep.tile([BS, 2 * U], f32, tag="zin_sb", name="zin_sb")
            nc.sync.dma_start(out=zin_sb[:, :], in_=z_ext[:, :])

            W_ms_sb = ep.tile([64, 256], f32, tag="W_ms_sb", name="W_ms_sb")
            nc.sync.dma_start(out=W_ms_sb[:, :], in_=W_ms_ext[:, :])
            W_rs_sb = ep.tile([128, 256], f32, tag="W_rs_sb", name="W_rs_sb")
            nc.sync.dma_start(out=W_rs_sb[:, :], in_=W_rs_ext[:, :])
            W_cmb_sb, W_mu_sb, W_b_sb = [], [], []
            for k in range(4):
                t = ep.tile([128, 512], f32, tag=f"W_cmb_sb{k}", name=f"W_cmb_sb{k}")
                nc.sync.dma_start(out=t[:, :], in_=W_cmb_ext[k * 128:(k + 1) * 128, :])
                W_cmb_sb.append(t)
                t = ep.tile([128, 512], f32, tag=f"W_mu_sb{k}", name=f"W_mu_sb{k}")
                nc.sync.dma_start(out=t[:, :], in_=W_mu_ext[k * 128:(k + 1) * 128, :])
                W_mu_sb.append(t)
                t = ep.tile([128, 512], f32, tag=f"W_b_sb{k}", name=f"W_b_sb{k}")
                nc.sync.dma_start(out=t[:, :], in_=W_b_ext[k * 128:(k + 1) * 128, :])
                W_b_sb.append(t)
            W_om_sb = ep.tile([128, 4], f32, tag="W_om_sb", name="W_om_sb")
            nc.sync.dma_start(out=W_om_sb[:, :],
                              in_=W_om_ext[:, :].rearrange("(k p) o -> p (k o)", p=128))


            tps = eps.tile([128, BS], f32, tag="etp", name="etp_min")
            nc.tensor.transpose(tps[0:64, :], min_sb[:, :], ident[0:BS, 0:BS])
            minT = ep.tile([64, BS], f32, tag="minT", name="minT")
            nc.scalar.copy(out=minT[:, :], in_=tps[0:64, :])

            tps = eps.tile([128, BS], f32, tag="etp", name="etp_rin")
            nc.tensor.transpose(tps[:, :], rin_sb[:, :], ident[0:BS, 0:BS])
            rinT = ep.tile([128, BS], f32, tag="rinT", name="rinT")
            nc.scalar.copy(out=rinT[:, :], in_=tps[:, :])

            # initial state z0 (flat [x-chunks | y-chunks])
            st0 = stp.tile([128, 256], f32, tag="st", name="st0")
            for c in range(8):
                tps = eps.tile([128, BS], f32, tag="etp", name=f"etp_z{c}")
                nc.tensor.transpose(tps[:, :], zin_sb[:, c * 128:(c + 1) * 128],
                                    ident[0:BS, 0:BS])
                nc.scalar.copy(out=st0[:, c * 32:(c + 1) * 32], in_=tps[:, :])

            # ms / rs (feature-major [128, 32] chunks)
            msT = ep.tile([128, 64], f32, tag="msT", name="msT")
            rsT = ep.tile([128, 64], f32, tag="rsT", name="rsT")
            for o in range(2):
                pe1 = eps.tile([128, BS], f32, tag="eacc", name=f"ems{o}")
                nc.tensor.matmul(pe1[:, :], W_ms_sb[:, o * 128:(o + 1) * 128],
                                 minT[:, :], start=True, stop=True)
                nc.scalar.activation(msT[:, o * 32:(o + 1) * 32], pe1[:, :],
                                     AF.Relu, bias=b_ms_t[:, o:o + 1])
                pe2 = eps.tile([128, BS], f32, tag="eacc", name=f"ers{o}")
                nc.tensor.matmul(pe2[:, :], W_rs_sb[:, o * 128:(o + 1) * 128],
                                 rinT[:, :], start=True, stop=True)
                nc.scalar.activation(rsT[:, o * 32:(o + 1) * 32], pe2[:, :],
                                     AF.Relu, bias=b_rs_t[:, o:o + 1])

            stin = [msT[:, 0:32], msT[:, 32:64], rsT[:, 0:32], rsT[:, 32:64]]
            stT = ep.tile([128, 128], f32, tag="stT", name="stT")
            for o in range(4):
                pe1 = eps.tile([128, BS], f32, tag="eacc", name=f"est{o}")
                for k in range(4):
                    nc.tensor.matmul(pe1[:, :], W_cmb_sb[k][:, o * 128:(o + 1) * 128],
                                     stin[k], start=(k == 0), stop=(k == 3))
                nc.scalar.activation(stT[:, o * 32:(o + 1) * 32], pe1[:, :],
                                     AF.Relu, bias=b_cmb_t[:, o:o + 1])

            # mu / bb (feature-major), plus batch-major copies for output
            muT = ep.tile([128, 128], f32, tag="muT", name="muT")
            bbT = ep.tile([128, 128], f32, tag="bbT", name="bbT")
            mu_out = ep.tile([BS, 512], f32, tag="mu_out", name="mu_out")
            bb_out = ep.tile([BS, 512], f32, tag="bb_out", name="bb_out")
            for o in range(4):
                pe1 = eps.tile([128, BS], f32, tag="eacc", name=f"emu{o}")
                for k in range(4):
                    nc.tensor.matmul(pe1[:, :], W_mu_sb[k][:, o * 128:(o + 1) * 128],
                                     stT[:, k * 32:(k + 1) * 32],
                                     start=(k == 0), stop=(k == 3))
                nc.scalar.activation(muT[:, o * 32:(o + 1) * 32], pe1[:, :],
                                     AF.Relu, bias=b_mu_t[:, o:o + 1])
                nc.scalar.activation(one_dtmu[:, o * 32:(o + 1) * 32],
                                     muT[:, o * 32:(o + 1) * 32],
                                     AF.Identity, scale=DT, bias=1.0)
                pe2 = eps.tile([128, BS], f32, tag="eacc", name=f"ebb{o}")
                for k in range(4):
                    nc.tensor.matmul(pe2[:, :], W_b_sb[k][:, o * 128:(o + 1) * 128],
                                     stT[:, k * 32:(k + 1) * 32],
                                     start=(k == 0), stop=(k == 3))
                nc.scalar.activation(bbT[:, o * 32:(o + 1) * 32], pe2[:, :],
                                     AF.Relu, bias=b_b_t[:, o:o + 1])
                nc.scalar.activation(dtbb[:, o * 32:(o + 1) * 32],
                                     bbT[:, o * 32:(o + 1) * 32],
                                     AF.Copy, scale=DT)
                # batch-major mu/bb for host output
                tps = eps.tile([128, 128], f32, tag="etp2", name=f"etp_mu{o}")
                nc.tensor.transpose(tps[0:BS, :], muT[:, o * 32:(o + 1) * 32],
                                    ident[:, :])
                nc.scalar.copy(out=mu_out[:, o * 128:(o + 1) * 128], in_=tps[0:BS, :])
                tps = eps.tile([128, 128], f32, tag="etp2", name=f"etp_bb{o}")
                nc.tensor.transpose(tps[0:BS, :], bbT[:, o * 32:(o + 1) * 32],
                                    ident[:, :])
                nc.scalar.copy(out=bb_out[:, o * 128:(o + 1) * 128], in_=tps[0:BS, :])
            nc.sync.dma_start(out=mu_ext[:, :], in_=mu_out[:, :])
            nc.sync.dma_start(out=bb_ext[:, :], in_=bb_out[:, :])

            # omega
            om_ps = eps.tile([1, BS], f32, tag="eom", name="eom")
            for k in range(4):
                nc.tensor.matmul(om_ps[:, :], W_om_sb[:, k:k + 1],
                                 stT[:, k * 32:(k + 1) * 32],
                                 start=(k == 0), stop=(k == 3))
            omegaT = ep.tile([1, BS], f32, tag="omegaT", name="omegaT")
            nc.scalar.activation(omegaT[:, :], om_ps[:, :], AF.Relu,
                                 bias=b_om_t[0:1, 0:1])
            nc.sync.dma_start(out=omega_ext[:, :].rearrange("b o -> o b"),
                              in_=omegaT[:, :])

            # omega broadcast across partitions (outer product with ones)
            ones_t = ep.tile([1, 128], f32, tag="ones_t", name="ones_t")
            nc.gpsimd.memset(ones_t[:, :], 1.0)
            ob_ps = eps.tile([128, BS], f32, tag="eacc", name="eob")
            nc.tensor.matmul(ob_ps[:, :], ones_t[:, :], omegaT[:, :],
                             start=True, stop=True)
            ob_sb = ep.tile([128, BS], f32, tag="ob_sb", name="ob_sb")
            nc.scalar.copy(out=ob_sb[:, :], in_=ob_ps[:, :])

            # dtw[p, c*32+b] = DT * (c*128 + p + 1) * omega[b]
            ki = ep.tile([128, 1], i32, tag="ki", name="ki")
            nc.gpsimd.iota(ki[:, :], pattern=[[1, 1]], base=0, channel_multiplier=1)
            kf = ep.tile([128, 1], f32, tag="kf", name="kf")
            nc.vector.tensor_copy(kf[:, :], ki[:, :])
            dtk = ep.tile([128, 4], f32, tag="dtk", name="dtk")
            for c in range(4):
                nc.vector.tensor_scalar(dtk[:, c:c + 1], kf[:, :],
                                        float(1 + 128 * c), DT, OP.add, OP.mult)
                nc.vector.tensor_scalar_mul(dtw[:, c * 32:(c + 1) * 32],
                                            ob_sb[:, :], dtk[:, c:c + 1])

        # ---------- fused hopf step helper (flat [128, 256] state) --------
        hp = ctx.enter_context(tc.tile_pool(name="hopf", bufs=2))
        XH, YH = slice(0, 128), slice(128, 256)

        def hopf_step(idx, st):
            sq = hp.tile([128, 256], f32, tag="sq", name=f"sq_{idx}")
            r2 = hp.tile([128, 128], f32, tag="r2", name=f"r2_{idx}")
            q1 = hp.tile([128, 128], f32, tag="q1", name=f"q1_{idx}")
            wy = hp.tile([128, 128], f32, tag="wy", name=f"wy_{idx}")
            vw = hp.tile([128, 256], f32, tag="vw", name=f"vw_{idx}")
            tq = hp.tile([128, 256], f32, tag="tq", name=f"tq_{idx}")
            stn = stp.tile([128, 256], f32, tag="st", name=f"st_{idx}")
            nc.scalar.activation(sq[:, :], st[:, :], AF.Square)
            nc.vector.tensor_add(r2[:, :], sq[:, XH], sq[:, YH])
            # q1 = 1 + DT*(mu - r2)
            nc.vector.scalar_tensor_tensor(q1[:, :], r2[:, :], -DT,
                                           one_dtmu[:, :], OP.mult, OP.add)
            nc.vector.tensor_mul(tq[:, XH], st[:, XH], q1[:, :])
            nc.vector.tensor_mul(tq[:, YH], st[:, YH], q1[:, :])
            nc.vector.tensor_mul(wy[:, :], dtw[:, :], st[:, YH])
            # vw = [DT*bb - DT*w*y | DT*w*x]
            nc.vector.scalar_tensor_tensor(vw[:, XH], wy[:, :], -1.0,
                                           dtbb[:, :], OP.mult, OP.add)
            nc.vector.tensor_mul(vw[:, YH], dtw[:, :], st[:, XH])
            nc.vector.tensor_add(stn[:, :], tq[:, :], vw[:, :])
            return stn

        # z1 (= z_out) and its batch-major copy
        st1 = hopf_step(0, st0)
        with tc.tile_pool(name="zoutp", bufs=1) as zop, \
             tc.tile_pool(name="zoutps", bufs=2, space="PSUM") as zops:
            zout_sb = zop.tile([BS, 2 * U], f32, tag="zout_sb", name="zout_sb")
            for c in range(8):
                tps = zops.tile([128, 128], f32, tag="ztp", name=f"ztp{c}")
                nc.tensor.transpose(tps[0:BS, :], st1[:, c * 32:(c + 1) * 32],
                                    ident[:, :])
                nc.scalar.copy(out=zout_sb[:, c * 128:(c + 1) * 128],
                               in_=tps[0:BS, :])
            nc.sync.dma_start(out=zout_ext[:, :], in_=zout_sb[:, :])

        # ---------- main pipeline ----------
        zp = ctx.enter_context(tc.tile_pool(name="zs", bufs=2))
        mp = ctx.enter_context(tc.tile_pool(name="mlp", bufs=1))
        m2p = ctx.enter_context(tc.tile_pool(name="mlp2", bufs=2))
        pp = ctx.enter_context(tc.tile_pool(name="mps", bufs=2, space="PSUM"))

        stc = st1
        zs_tiles = {}

        def zgen(g):
            nonlocal stc
            # zs[p, c, sl*32+b]: c in 0..3 = x feature chunks, 4..7 = y chunks
            zs = zp.tile([128, 8, G], f32r, tag="zs", name=f"zs{g}")
            zs_tiles[g] = zs
            for sl in range(G_STEPS):
                idx = g * G_STEPS + sl
                if idx > 0:
                    stc = hopf_step(idx, stc)
                nc.scalar.copy(
                    out=zs[:, :, sl * 32:(sl + 1) * 32],
                    in_=stc[:, :].rearrange("p (c b) -> p c b", b=32))

        zgen(0)
        for g in range(NG):
            if g + 1 < NG:
                zgen(g + 1)
            zs = zs_tiles.pop(g)

            def xin(k):
                return zs[:, k, :]

            def yin(k):
                return zs[:, 4 + k, :]

            # --- layer 0: [B,1024] -> yr0, yi0 [128 x 8 x G] ---
            yr0 = mp.tile([128, 8, G], f32r, tag="yr0", name=f"yr0_{g}")
            yi0 = mp.tile([128, 8, G], f32r, tag="yi0", name=f"yi0_{g}")
            for o in range(8):
                osl = slice(o * 128, (o + 1) * 128)
                t1 = pp.tile([128, G], f32, tag="t1", name=f"t1_l0_{g}_{o}")
                t2 = pp.tile([128, G], f32, tag="t2", name=f"t2_l0_{g}_{o}")
                for k in range(4):
                    nc.tensor.matmul(t1[:, :], wr0[k][:, osl], xin(k),
                                     start=(k == 0), stop=False)
                    nc.tensor.matmul(t2[:, :], wr0[k][:, osl], yin(k),
                                     start=(k == 0), stop=False)
                for k in range(4):
                    nc.tensor.matmul(t1[:, :], nwi0[k][:, osl], yin(k),
                                     start=False, stop=(k == 3))
                    nc.tensor.matmul(t2[:, :], wi0[k][:, osl], xin(k),
                                     start=False, stop=(k == 3))
                nc.scalar.activation(yr0[:, o, :], t1[:, :], AF.Relu,
                                     bias=br0_t[:, o:o + 1])
                nc.scalar.activation(yi0[:, o, :], t2[:, :], AF.Relu,
                                     bias=bi0_t[:, o:o + 1])

            # --- layer 1 (Karatsuba: t1=Wr@xr, t2=Wi@xi, t3=(Wr+Wi)@(xr+xi),
            #     yr1 = relu(t1-t2+b), yi1 = relu(t3-t1-t2+b)) ---
            xs1 = mp.tile([128, 8, G], f32r, tag="xs1", name=f"xs1_{g}")
            nc.vector.tensor_add(xs1[:, :, :], yr0[:, :, :], yi0[:, :, :])
            yr1 = mp.tile([128, 4, G], f32r, tag="yr1", name=f"yr1_{g}")
            yi1 = mp.tile([128, 4, G], f32r, tag="yi1", name=f"yi1_{g}")
            for o in range(4):
                osl = slice(o * 128, (o + 1) * 128)
                t1 = pp.tile([128, G], f32, tag="t1", name=f"t1_l1_{g}_{o}")
                t2 = pp.tile([128, G], f32, tag="t2", name=f"t2_l1_{g}_{o}")
                t3 = pp.tile([128, G], f32, tag="t3", name=f"t3_l1_{g}_{o}")
                for k in range(8):
                    nc.tensor.matmul(t1[:, :], wr1[k][:, osl], yr0[:, k, :],
                                     start=(k == 0), stop=(k == 7))
                    nc.tensor.matmul(t2[:, :], wi1[k][:, osl], yi0[:, k, :],
                                     start=(k == 0), stop=(k == 7))
                    nc.tensor.matmul(t3[:, :], wq1[k][:, osl], xs1[:, k, :],
                                     start=(k == 0), stop=(k == 7))
                c1 = mp.tile([128, G], f32, tag="c1", name=f"c1_{g}_{o}")
                nc.vector.tensor_copy(c1[:, :], t1[:, :])
                d = mp.tile([128, G], f32, tag="d", name=f"d_{g}_{o}")
                nc.vector.tensor_sub(d[:, :], c1[:, :], t2[:, :])
                nc.scalar.activation(yr1[:, o, :], d[:, :], AF.Relu,
                                     bias=br1_t[:, o:o + 1])
                f = mp.tile([128, G], f32, tag="f", name=f"f_{g}_{o}")
                nc.vector.tensor_sub(f[:, :], t3[:, :], c1[:, :])
                e = mp.tile([128, G], f32, tag="e", name=f"e_{g}_{o}")
                nc.vector.tensor_sub(e[:, :], f[:, :], t2[:, :])
                nc.scalar.activation(yi1[:, o, :], e[:, :], AF.Relu,
                                     bias=bi1_t[:, o:o + 1])

            # --- layer 2 (real part only): [B,1024] -> yr2 [128 x 2 x G] ---
            yr2 = m2p.tile([128, 2, G], f32, tag="yr2", name=f"yr2_{g}")
            for o in range(2):
                osl = slice(o * 128, (o + 1) * 128)
                t1 = pp.tile([128, G], f32, tag="t1", name=f"t1_l2_{g}_{o}")
                for k in range(4):
                    nc.tensor.matmul(t1[:, :], wr2[k][:, osl], yr1[:, k, :],
                                     start=(k == 0), stop=False)
                for k in range(4):
                    nc.tensor.matmul(t1[:, :], nwi2[k][:, osl], yi1[:, k, :],
                                     start=False, stop=(k == 3))
                nc.scalar.activation(yr2[:, o, :], t1[:, :], AF.Relu,
                                     bias=br2_t[:, o:o + 1])

            # --- DMA out feature-major (contiguous); host transposes ---
            nc.sync.dma_start(out=out_ext[g, :, :, :], in_=yr2[:, :, :])

    nc.compile()
    return nc


def _get_nc():
    if "nc" not in _CACHE:
        _CACHE["nc"] = _build()
    return _CACHE["nc"]


def kernel(**inputs):
    from concourse.bass_utils import run_bass_kernel_spmd

    nc = _get_nc()
    inp = {k: np.ascontiguousarray(np.asarray(v, dtype=np.float32))
           for k, v in inputs.items()}
    shard_keys = ("motion_state", "robot_state", "z")
    in_maps = []
    for i in range(NCORES):
        m = {}
        for k, v in inp.items():
            if k in shard_keys:
                m[k] = np.ascontiguousarray(v[i * BS:(i + 1) * BS])
            else:
                m[k] = v
        in_maps.append(m)
    res = run_bass_kernel_spmd(nc, in_maps, core_ids=list(range(NCORES)))
    outs = res.results

    def gather(name):
        return np.concatenate([outs[i][name] for i in range(NCORES)], axis=0)

    # out_fm[g, p, o, sl*BS + b] -> out[b, g*G_STEPS + sl, o*128 + p], per core
    parts = []
    for i in range(NCORES):
        fm = outs[i]["out_fm"].reshape(NG, 128, 2, G_STEPS, BS)
        full = fm.transpose(4, 0, 3, 2, 1).reshape(BS, NG * G_STEPS, A)
        parts.append(np.ascontiguousarray(full[:, :STEPS, :]))
    out = np.concatenate(parts, axis=0)
    return (out, gather("z_out"), gather("omega"),
            gather("mu"), gather("bb"))


# revision 37
# speedup vs baseline: 1.0437x; 1.0437x over previous
"""Trainium2 Bass kernel for nn_Actor (Hopf-oscillator CPG actor network).

Strategy
--------
The 100-step recurrence only advances z via the cheap elementwise Hopf update;
the 3-layer complex MLP applied at each step does NOT feed back into the
recurrence.  So per core (data-parallel batch shard of 32):

  1. Encoder (exact fp32 matmuls, feature-major layout, tiny).
  2. Sequentially generate z_1..z_100 on the Vector/Scalar engines
     (state kept feature-major, packed flat [128, 256] = [x-chunks | y-chunks]).
  3. Run the complex MLP over groups of 10 steps (320 rows) as large
     weight-stationary float32r matmuls on the Tensor engine.  Complex
     products are accumulated directly in PSUM using pre-negated weight
     copies (yr = Wr@x + (-Wi)@y, yi = Wi@x + Wr@y), so the combine is a
     single ScalarE relu+bias from PSUM (layer 1 uses the 3-multiply
     Karatsuba form with VectorE combines instead).  Only the real part of
     layer 2 is computed (the reference keeps out[:, :, :256]).
  4. Results are DMA'd out feature-major; the host transposes to
     [B, STEPS, A] while unsharding.

All 8 cores run the same SPMD graph on their own batch shard; weights are
replicated; no collectives.  abs(relu(x)) == relu(x) so the encoder's abs()
is dropped.
"""

import os
import sys

import numpy as np

for _p in ("/opt/trn_rl_repo", "/opt/pypackages"):
    if os.path.isdir(_p) and _p not in sys.path:
        sys.path.append(_p)

B = 256
NCORES = 8
BS = B // NCORES          # 32 batch rows per core
U = 512                   # oscillator units
STEPS = 100
DT = 0.001
A = 256                   # action dim

G_STEPS = 10              # steps per MLP group
G = G_STEPS * BS          # 320 rows per group (moving dim; >=256 keeps f32r fast)
NG = 10                   # 10 * 10 = 100 generated states, zero padding

_CACHE = {}


def _build():
    from contextlib import ExitStack

    from concourse import bacc, masks, mybir

    from concourse.tile import TileContext

    f32 = mybir.dt.float32
    f32r = mybir.dt.float32r
    i32 = mybir.dt.int32
    AF = mybir.ActivationFunctionType
    OP = mybir.AluOpType

    nc = bacc.Bacc("TRN2", target_bir_lowering=False, debug=False,
                   num_devices=NCORES)

    def dp(name, shape, out=False):
        return nc.declare_dram_parameter(name, list(shape), f32, isOutput=out)

    motion_ext = dp("motion_state", [BS, 64])
    robot_ext = dp("robot_state", [BS, 128])
    z_ext = dp("z", [BS, 2 * U])
    W_ms_ext = dp("W_ms", [64, 256]); b_ms_ext = dp("b_ms", [256])
    W_rs_ext = dp("W_rs", [128, 256]); b_rs_ext = dp("b_rs", [256])
    W_cmb_ext = dp("W_cmb", [512, 512]); b_cmb_ext = dp("b_cmb", [512])
    W_om_ext = dp("W_om", [512, 1]); b_om_ext = dp("b_om", [1])
    W_mu_ext = dp("W_mu", [512, 512]); b_mu_ext = dp("b_mu", [512])
    W_b_ext = dp("W_b", [512, 512]); b_b_ext = dp("b_b", [512])
    Wr0_ext = dp("Wr0", [512, 1024]); Wi0_ext = dp("Wi0", [512, 1024])
    br0_ext = dp("br0", [1024]); bi0_ext = dp("bi0", [1024])
    Wr1_ext = dp("Wr1", [1024, 512]); Wi1_ext = dp("Wi1", [1024, 512])
    br1_ext = dp("br1", [512]); bi1_ext = dp("bi1", [512])
    Wr2_ext = dp("Wr2", [512, 256]); Wi2_ext = dp("Wi2", [512, 256])
    br2_ext = dp("br2", [256]); bi2_ext = dp("bi2", [256])

    # feature-major, group-major output; host transposes during unshard
    out_ext = dp("out_fm", [NG, 128, 2, G], out=True)
    zout_ext = dp("z_out", [BS, 2 * U], out=True)
    omega_ext = dp("omega", [BS, 1], out=True)
    mu_ext = dp("mu", [BS, U], out=True)
    bb_ext = dp("bb", [BS, U], out=True)

    with TileContext(nc) as tc, ExitStack() as ctx:
        # ---------- persistent pools ----------
        wp = ctx.enter_context(tc.tile_pool(name="wpool", bufs=1))
        stp = ctx.enter_context(tc.tile_pool(name="stpool", bufs=2))

        ident = wp.tile([128, 128], f32, tag="ident", name="ident")
        masks.make_identity(nc, ident[:, :])

        # MLP weights (float32r, cast on DMA by gpsimd) + negated Wi copies
        def wload(ext, n_k, width, base):
            ts = []
            for k in range(n_k):
                t = wp.tile([128, width], f32r, tag=f"{base}{k}", name=f"{base}{k}")
                nc.gpsimd.dma_start(out=t[:, :], in_=ext[k * 128:(k + 1) * 128, :])
                ts.append(t)
            return ts

        wr0 = wload(Wr0_ext, 4, 1024, "wr0_")
        wi0 = wload(Wi0_ext, 4, 1024, "wi0_")
        wr1 = wload(Wr1_ext, 8, 512, "wr1_")
        wi1 = wload(Wi1_ext, 8, 512, "wi1_")
        wr2 = wload(Wr2_ext, 4, 256, "wr2_")
        wi2 = wload(Wi2_ext, 4, 256, "wi2_")
        nwi0, nwi2 = [], []
        for k in range(4):
            t = wp.tile([128, 1024], f32r, tag=f"nwi0_{k}", name=f"nwi0_{k}")
            nc.vector.tensor_scalar_mul(t[:, :], wi0[k][:, :], -1.0)
            nwi0.append(t)
        wq1 = []
        for k in range(8):
            t = wp.tile([128, 512], f32r, tag=f"wq1_{k}", name=f"wq1_{k}")
            nc.vector.tensor_add(t[:, :], wr1[k][:, :], wi1[k][:, :])
            wq1.append(t)
        for k in range(4):
            t = wp.tile([128, 256], f32r, tag=f"nwi2_{k}", name=f"nwi2_{k}")
            nc.vector.tensor_scalar_mul(t[:, :], wi2[k][:, :], -1.0)
            nwi2.append(t)

        # biases, partition-major: tile[p, o] = b[o*128 + p]
        def bias_tile(ext, n_chunks, name, eng):
            t = wp.tile([128, n_chunks], f32, tag=name, name=name)
            eng.dma_start(out=t[:, :],
                          in_=ext[:].rearrange("(o p) -> p o", p=128))
            return t

        br0_t = bias_tile(br0_ext, 8, "br0_t", nc.sync)
        bi0_t = bias_tile(bi0_ext, 8, "bi0_t", nc.sync)
        br1_t = bias_tile(br1_ext, 4, "br1_t", nc.sync)
        bi1_t = bias_tile(bi1_ext, 4, "bi1_t", nc.sync)
        br2_t = bias_tile(br2_ext, 2, "br2_t", nc.sync)
        b_ms_t = bias_tile(b_ms_ext, 2, "b_ms_t", nc.sync)
        b_rs_t = bias_tile(b_rs_ext, 2, "b_rs_t", nc.sync)
        b_cmb_t = bias_tile(b_cmb_ext, 4, "b_cmb_t", nc.sync)
        b_mu_t = bias_tile(b_mu_ext, 4, "b_mu_t", nc.sync)
        b_b_t = bias_tile(b_b_ext, 4, "b_b_t", nc.sync)
        b_om_t = wp.tile([1, 1], f32, tag="b_om_t", name="b_om_t")
        nc.sync.dma_start(out=b_om_t[:, :],
                          in_=b_om_ext[:].rearrange("(a b) -> a b", b=1))

        # hopf constants (flat [128, 128]: col = chunk*32 + batch)
        one_dtmu = wp.tile([128, 128], f32, tag="one_dtmu", name="one_dtmu")
        dtbb = wp.tile([128, 128], f32, tag="dtbb", name="dtbb")
        dtw = wp.tile([128, 128], f32, tag="dtw", name="dtw")

        # ---------- encoder (scratch pools, freed before the main loop) ----
        with tc.tile_pool(name="encp", bufs=1) as ep, \
             tc.tile_pool(name="encps", bufs=2, space="PSUM") as eps:

            # inputs first (everything downstream depends on them)
            min_sb = ep.tile([BS, 64], f32, tag="min_sb", name="min_sb")
            nc.sync.dma_start(out=min_sb[:, :], in_=motion_ext[:, :])
            rin_sb = ep.tile([BS, 128], f32, tag="rin_sb", name="rin_sb")
            nc.sync.dma_start(out=rin_sb[:, :], in_=robot_ext[:, :])
            zin_sb = ep.tile([BS, 2 * U], f32, tag="zin_sb", name="zin_sb")
            nc.sync.dma_start(out=zin_sb[:, :], in_=z_ext[:, :])

            W_ms_sb = ep.tile([64, 256], f32, tag="W_ms_sb", name="W_ms_sb")
            nc.sync.dma_start(out=W_ms_sb[:, :], in_=W_ms_ext[:, :])
            W_rs_sb = ep.tile([128, 256], f32, tag="W_rs_sb", name="W_rs_sb")
            nc.sync.dma_start(out=W_rs_sb[:, :], in_=W_rs_ext[:, :])
            W_cmb_sb, W_mu_sb, W_b_sb = [], [], []
            for k in range(4):
                t = ep.tile([128, 512], f32, tag=f"W_cmb_sb{k}", name=f"W_cmb_sb{k}")
                nc.sync.dma_start(out=t[:, :], in_=W_cmb_ext[k * 128:(k + 1) * 128, :])
                W_cmb_sb.append(t)
                t = ep.tile([128, 512], f32, tag=f"W_mu_sb{k}", name=f"W_mu_sb{k}")
                nc.sync.dma_start(out=t[:, :], in_=W_mu_ext[k * 128:(k + 1) * 128, :])
                W_mu_sb.append(t)
                t = ep.tile([128, 512], f32, tag=f"W_b_sb{k}", name=f"W_b_sb{k}")
                nc.sync.dma_start(out=t[:, :], in_=W_b_ext[k * 128:(k + 1) * 128, :])
                W_b_sb.append(t)
            W_om_sb = ep.tile([128, 4], f32, tag="W_om_sb", name="W_om_sb")
            nc.sync.dma_start(out=W_om_sb[:, :],
                              in_=W_om_ext[:, :].rearrange("(k p) o -> p (k o)", p=128))


            tps = eps.tile([128, BS], f32, tag="etp", name="etp_min")
            nc.tensor.transpose(tps[0:64, :], min_sb[:, :], ident[0:BS, 0:BS])
            minT = ep.tile([64, BS], f32, tag="minT", name="minT")
            nc.scalar.copy(out=minT[:, :], in_=tps[0:64, :])

            tps = eps.tile([128, BS], f32, tag="etp", name="etp_rin")
            nc.tensor.transpose(tps[:, :], rin_sb[:, :], ident[0:BS, 0:BS])
            rinT = ep.tile([128, BS], f32, tag="rinT", name="rinT")
            nc.scalar.copy(out=rinT[:, :], in_=tps[:, :])

            # initial state z0 (flat [x-chunks | y-chunks])
            st0 = stp.tile([128, 256], f32, tag="st", name="st0")
            for c in range(8):
                tps = eps.tile([128, BS], f32, tag="etp", name=f"etp_z{c}")
                nc.tensor.transpose(tps[:, :], zin_sb[:, c * 128:(c + 1) * 128],
                                    ident[0:BS, 0:BS])
                nc.scalar.copy(out=st0[:, c * 32:(c + 1) * 32], in_=tps[:, :])

            # ms / rs (feature-major [128, 32] chunks)
            msT = ep.tile([128, 64], f32, tag="msT", name="msT")
            rsT = ep.tile([128, 64], f32, tag="rsT", name="rsT")
            for o in range(2):
                pe1 = eps.tile([128, BS], f32, tag="eacc", name=f"ems{o}")
                nc.tensor.matmul(pe1[:, :], W_ms_sb[:, o * 128:(o + 1) * 128],
                                 minT[:, :], start=True, stop=True)
                nc.scalar.activation(msT[:, o * 32:(o + 1) * 32], pe1[:, :],
                                     AF.Relu, bias=b_ms_t[:, o:o + 1])
                pe2 = eps.tile([128, BS], f32, tag="eacc", name=f"ers{o}")
                nc.tensor.matmul(pe2[:, :], W_rs_sb[:, o * 128:(o + 1) * 128],
                                 rinT[:, :], start=True, stop=True)
                nc.scalar.activation(rsT[:, o * 32:(o + 1) * 32], pe2[:, :],
                                     AF.Relu, bias=b_rs_t[:, o:o + 1])

            stin = [msT[:, 0:32], msT[:, 32:64], rsT[:, 0:32], rsT[:, 32:64]]
            stT = ep.tile([128, 128], f32, tag="stT", name="stT")
            for o in range(4):
                pe1 = eps.tile([128, BS], f32, tag="eacc", name=f"est{o}")
                for k in range(4):
                    nc.tensor.matmul(pe1[:, :], W_cmb_sb[k][:, o * 128:(o + 1) * 128],
                                     stin[k], start=(k == 0), stop=(k == 3))
                nc.scalar.activation(stT[:, o * 32:(o + 1) * 32], pe1[:, :],
                                     AF.Relu, bias=b_cmb_t[:, o:o + 1])

            # mu / bb (feature-major), plus batch-major copies for output
            muT = ep.tile([128, 128], f32, tag="muT", name="muT")
            bbT = ep.tile([128, 128], f32, tag="bbT", name="bbT")
            mu_out = ep.tile([BS, 512], f32, tag="mu_out", name="mu_out")
            bb_out = ep.tile([BS, 512], f32, tag="bb_out", name="bb_out")
            for o in range(4):
                pe1 = eps.tile([128, BS], f32, tag="eacc", name=f"emu{o}")
                for k in range(4):
                    nc.tensor.matmul(pe1[:, :], W_mu_sb[k][:, o * 128:(o + 1) * 128],
                                     stT[:, k * 32:(k + 1) * 32],
                                     start=(k == 0), stop=(k == 3))
                nc.scalar.activation(muT[:, o * 32:(o + 1) * 32], pe1[:, :],
                                     AF.Relu, bias=b_mu_t[:, o:o + 1])
                nc.scalar.activation(one_dtmu[:, o * 32:(o + 1) * 32],
                                     muT[:, o * 32:(o + 1) * 32],
                                     AF.Identity, scale=DT, bias=1.0)
                pe2 = eps.tile([128, BS], f32, tag="eacc", name=f"ebb{o}")
                for k in range(4):
                    nc.tensor.matmul(pe2[:, :], W_b_sb[k][:, o * 128:(o + 1) * 128],
                                     stT[:, k * 32:(k + 1) * 32],
                                     start=(k == 0), stop=(k == 3))
                nc.scalar.activation(bbT[:, o * 32:(o + 1) * 32], pe2[:, :],
                                     AF.Relu, bias=b_b_t[:, o:o + 1])
                nc.scalar.activation(dtbb[:, o * 32:(o + 1) * 32],
                                     bbT[:, o * 32:(o + 1) * 32],
                                     AF.Copy, scale=DT)
                # batch-major mu/bb for host output
                tps = eps.tile([128, 128], f32, tag="etp2", name=f"etp_mu{o}")
                nc.tensor.transpose(tps[0:BS, :], muT[:, o * 32:(o + 1) * 32],
                                    ident[:, :])
                nc.scalar.copy(out=mu_out[:, o * 128:(o + 1) * 128], in_=tps[0:BS, :])
                tps = eps.tile([128, 128], f32, tag="etp2", name=f"etp_bb{o}")
                nc.tensor.transpose(tps[0:BS, :], bbT[:, o * 32:(o + 1) * 32],
                                    ident[:, :])
                nc.scalar.copy(out=bb_out[:, o * 128:(o + 1) * 128], in_=tps[0:BS, :])
            nc.sync.dma_start(out=mu_ext[:, :], in_=mu_out[:, :])
            nc.sync.dma_start(out=bb_ext[:, :], in_=bb_out[:, :])

            # omega
            om_ps = eps.tile([1, BS], f32, tag="eom", name="eom")
            for k in range(4):
                nc.tensor.matmul(om_ps[:, :], W_om_sb[:, k:k + 1],
                                 stT[:, k * 32:(k + 1) * 32],
                                 start=(k == 0), stop=(k == 3))
            omegaT = ep.tile([1, BS], f32, tag="omegaT", name="omegaT")
            nc.scalar.activation(omegaT[:, :], om_ps[:, :], AF.Relu,
                                 bias=b_om_t[0:1, 0:1])
            nc.sync.dma_start(out=omega_ext[:, :].rearrange("b o -> o b"),
                              in_=omegaT[:, :])

            # omega broadcast across partitions (outer product with ones)
            ones_t = ep.tile([1, 128], f32, tag="ones_t", name="ones_t")
            nc.gpsimd.memset(ones_t[:, :], 1.0)
            ob_ps = eps.tile([128, BS], f32, tag="eacc", name="eob")
            nc.tensor.matmul(ob_ps[:, :], ones_t[:, :], omegaT[:, :],
                             start=True, stop=True)
            ob_sb = ep.tile([128, BS], f32, tag="ob_sb", name="ob_sb")
            nc.scalar.copy(out=ob_sb[:, :], in_=ob_ps[:, :])

            # dtw[p, c*32+b] = DT * (c*128 + p + 1) * omega[b]
            ki = ep.tile([128, 1], i32, tag="ki", name="ki")
            nc.gpsimd.iota(ki[:, :], pattern=[[1, 1]], base=0, channel_multiplier=1)
            kf = ep.tile([128, 1], f32, tag="kf", name="kf")
            nc.vector.tensor_copy(kf[:, :], ki[:, :])
            dtk = ep.tile([128, 4], f32, tag="dtk", name="dtk")
            for c in range(4):
                nc.vector.tensor_scalar(dtk[:, c:c + 1], kf[:, :],
                                        float(1 + 128 * c), DT, OP.add, OP.mult)
                nc.vector.tensor_scalar_mul(dtw[:, c * 32:(c + 1) * 32],
                                            ob_sb[:, :], dtk[:, c:c + 1])

        # ---------- fused hopf step helper (flat [128, 256] state) --------
        hp = ctx.enter_context(tc.tile_pool(name="hopf", bufs=2))
        XH, YH = slice(0, 128), slice(128, 256)

        def hopf_step(idx, st):
            sq = hp.tile([128, 256], f32, tag="sq", name=f"sq_{idx}")
            r2 = hp.tile([128, 128], f32, tag="r2", name=f"r2_{idx}")
            q1 = hp.tile([128, 128], f32, tag="q1", name=f"q1_{idx}")
            wy = hp.tile([128, 128], f32, tag="wy", name=f"wy_{idx}")
            vw = hp.tile([128, 256], f32, tag="vw", name=f"vw_{idx}")
            tq = hp.tile([128, 256], f32, tag="tq", name=f"tq_{idx}")
            stn = stp.tile([128, 256], f32, tag="st", name=f"st_{idx}")
            nc.scalar.activation(sq[:, :], st[:, :], AF.Square)
            nc.vector.tensor_add(r2[:, :], sq[:, XH], sq[:, YH])
            # q1 = 1 + DT*(mu - r2)
            nc.vector.scalar_tensor_tensor(q1[:, :], r2[:, :], -DT,
                                           one_dtmu[:, :], OP.mult, OP.add)
            nc.vector.tensor_mul(tq[:, XH], st[:, XH], q1[:, :])
            nc.vector.tensor_mul(tq[:, YH], st[:, YH], q1[:, :])
            nc.vector.tensor_mul(wy[:, :], dtw[:, :], st[:, YH])
            # vw = [DT*bb - DT*w*y | DT*w*x]
            nc.vector.scalar_tensor_tensor(vw[:, XH], wy[:, :], -1.0,
                                           dtbb[:, :], OP.mult, OP.add)
            nc.vector.tensor_mul(vw[:, YH], dtw[:, :], st[:, XH])
            nc.vector.tensor_add(stn[:, :], tq[:, :], vw[:, :])
            return stn

        # z1 (= z_out) and its batch-major copy
        st1 = hopf_step(0, st0)
        with tc.tile_pool(name="zoutp", bufs=1) as zop, \
             tc.tile_pool(name="zoutps", bufs=2, space="PSUM") as zops:
            zout_sb = zop.tile([BS, 2 * U], f32, tag="zout_sb", name="zout_sb")
            for c in range(8):
                tps = zops.tile([128, 128], f32, tag="ztp", name=f"ztp{c}")
                nc.tensor.transpose(tps[0:BS, :], st1[:, c * 32:(c + 1) * 32],
                                    ident[:, :])
                nc.scalar.copy(out=zout_sb[:, c * 128:(c + 1) * 128],
                               in_=tps[0:BS, :])
            nc.sync.dma_start(out=zout_ext[:, :], in_=zout_sb[:, :])

        # ---------- main pipeline ----------
        zp = ctx.enter_context(tc.tile_pool(name="zs", bufs=2))
        mp = ctx.enter_context(tc.tile_pool(name="mlp", bufs=1))
        m2p = ctx.enter_context(tc.tile_pool(name="mlp2", bufs=2))
        pp = ctx.enter_context(tc.tile_pool(name="mps", bufs=2, space="PSUM"))

        stc = st1
        zs_tiles = {}

        def zgen(g):
            nonlocal stc
            # zs[p, c, sl*32+b]: c in 0..3 = x feature chunks, 4..7 = y chunks
            zs = zp.tile([128, 8, G], f32r, tag="zs", name=f"zs{g}")
            zs_tiles[g] = zs
            for sl in range(G_STEPS):
                idx = g * G_STEPS + sl
                if idx > 0:
                    stc = hopf_step(idx, stc)
                nc.scalar.copy(
                    out=zs[:, :, sl * 32:(sl + 1) * 32],
                    in_=stc[:, :].rearrange("p (c b) -> p c b", b=32))

        zgen(0)
        for g in range(NG):
            if g + 1 < NG:
                zgen(g + 1)
            zs = zs_tiles.pop(g)

            def xin(k):
                return zs[:, k, :]

            def yin(k):
                return zs[:, 4 + k, :]

            # --- layer 0: [B,1024] -> yr0, yi0 [128 x 8 x G] ---
            yr0 = mp.tile([128, 8, G], f32r, tag="yr0", name=f"yr0_{g}")
            yi0 = mp.tile([128, 8, G], f32r, tag="yi0", name=f"yi0_{g}")
            for o in range(8):
                osl = slice(o * 128, (o + 1) * 128)
                t1 = pp.tile([128, G], f32, tag="t1", name=f"t1_l0_{g}_{o}")
                t2 = pp.tile([128, G], f32, tag="t2", name=f"t2_l0_{g}_{o}")
                for k in range(4):
                    nc.tensor.matmul(t1[:, :], wr0[k][:, osl], xin(k),
                                     start=(k == 0), stop=False)
                    nc.tensor.matmul(t2[:, :], wr0[k][:, osl], yin(k),
                                     start=(k == 0), stop=False)
                for k in range(4):
                    nc.tensor.matmul(t1[:, :], nwi0[k][:, osl], yin(k),
                                     start=False, stop=(k == 3))
                    nc.tensor.matmul(t2[:, :], wi0[k][:, osl], xin(k),
                                     start=False, stop=(k == 3))
                nc.scalar.activation(yr0[:, o, :], t1[:, :], AF.Relu,
                                     bias=br0_t[:, o:o + 1])
                nc.scalar.activation(yi0[:, o, :], t2[:, :], AF.Relu,
                                     bias=bi0_t[:, o:o + 1])

            # --- layer 1 (Karatsuba: t1=Wr@xr, t2=Wi@xi, t3=(Wr+Wi)@(xr+xi),
            #     yr1 = relu(t1-t2+b), yi1 = relu(t3-t1-t2+b)) ---
            xs1 = mp.tile([128, 8, G], f32r, tag="xs1", name=f"xs1_{g}")
            nc.vector.tensor_add(xs1[:, :, :], yr0[:, :, :], yi0[:, :, :])
            yr1 = mp.tile([128, 4, G], f32r, tag="yr1", name=f"yr1_{g}")
            yi1 = mp.tile([128, 4, G], f32r, tag="yi1", name=f"yi1_{g}")
            for o in range(4):
                osl = slice(o * 128, (o + 1) * 128)
                t1 = pp.tile([128, G], f32, tag="t1", name=f"t1_l1_{g}_{o}")
                t2 = pp.tile([128, G], f32, tag="t2", name=f"t2_l1_{g}_{o}")
                t3 = pp.tile([128, G], f32, tag="t3", name=f"t3_l1_{g}_{o}")
                for k in range(8):
                    nc.tensor.matmul(t1[:, :], wr1[k][:, osl], yr0[:, k, :],
                                     start=(k == 0), stop=(k == 7))
                    nc.tensor.matmul(t2[:, :], wi1[k][:, osl], yi0[:, k, :],
                                     start=(k == 0), stop=(k == 7))
                    nc.tensor.matmul(t3[:, :], wq1[k][:, osl], xs1[:, k, :],
                                     start=(k == 0), stop=(k == 7))
                c1 = mp.tile([128, G], f32, tag="c1", name=f"c1_{g}_{o}")
                nc.vector.tensor_copy(c1[:, :], t1[:, :])
                d = mp.tile([128, G], f32, tag="d", name=f"d_{g}_{o}")
                nc.vector.tensor_sub(d[:, :], c1[:, :], t2[:, :])
                nc.scalar.activation(yr1[:, o, :], d[:, :], AF.Relu,
                                     bias=br1_t[:, o:o + 1])
                f = mp.tile([128, G], f32, tag="f", name=f"f_{g}_{o}")
                nc.vector.tensor_sub(f[:, :], t3[:, :], c1[:, :])
                e = mp.tile([128, G], f32, tag="e", name=f"e_{g}_{o}")
                nc.vector.tensor_sub(e[:, :], f[:, :], t2[:, :])
                nc.scalar.activation(yi1[:, o, :], e[:, :], AF.Relu,
                                     bias=bi1_t[:, o:o + 1])

            # --- layer 2 (real part only): [B,1024] -> yr2 [128 x 2 x G] ---
            yr2 = m2p.tile([128, 2, G], f32, tag="yr2", name=f"yr2_{g}")
            for o in range(2):
                osl = slice(o * 128, (o + 1) * 128)
                t1 = pp.tile([128, G], f32, tag="t1", name=f"t1_l2_{g}_{o}")
                for k in range(4):
                    nc.tensor.matmul(t1[:, :], wr2[k][:, osl], yr1[:, k, :],
                                     start=(k == 0), stop=False)
                for k in range(4):
                    nc.tensor.matmul(t1[:, :], nwi2[k][:, osl], yi1[:, k, :],
                                     start=False, stop=(k == 3))
                nc.scalar.activation(yr2[:, o, :], t1[:, :], AF.Relu,
                                     bias=br2_t[:, o:o + 1])

            # --- DMA out feature-major (contiguous); host transposes ---
            nc.sync.dma_start(out=out_ext[g, :, :, :], in_=yr2[:, :, :])

    nc.compile()
    return nc


def _get_nc():
    if "nc" not in _CACHE:
        _CACHE["nc"] = _build()
    return _CACHE["nc"]


def kernel(**inputs):
    from concourse.bass_utils import run_bass_kernel_spmd

    nc = _get_nc()
    inp = {k: np.ascontiguousarray(np.asarray(v, dtype=np.float32))
           for k, v in inputs.items()}
    shard_keys = ("motion_state", "robot_state", "z")
    in_maps = []
    for i in range(NCORES):
        m = {}
        for k, v in inp.items():
            if k in shard_keys:
                m[k] = np.ascontiguousarray(v[i * BS:(i + 1) * BS])
            else:
                m[k] = v
        in_maps.append(m)
    res = run_bass_kernel_spmd(nc, in_maps, core_ids=list(range(NCORES)))
    outs = res.results

    def gather(name):
        return np.concatenate([outs[i][name] for i in range(NCORES)], axis=0)

    # out_fm[g, p, o, sl*BS + b] -> out[b, g*G_STEPS + sl, o*128 + p], per core
    parts = []
    for i in range(NCORES):
        fm = outs[i]["out_fm"].reshape(NG, 128, 2, G_STEPS, BS)
        full = fm.transpose(4, 0, 3, 2, 1).reshape(BS, NG * G_STEPS, A)
        parts.append(np.ascontiguousarray(full[:, :STEPS, :]))
    out = np.concatenate(parts, axis=0)
    return (out, gather("z_out"), gather("omega"),
            gather("mu"), gather("bb"))
